# revision 4
# baseline (speedup 1.0000x reference)
"""Trainium2 Bass kernel for nn_CRATE (12-layer CRATE-style transformer).

Sharding over 8 NeuronCores: 4 batch groups x 2-way parity-interleaved
sequence split.  Core c handles batch b=c//2 and parity half=c%2: it owns
absolute rows {2*j + half, j=0..511}.  With this split both halves have an
IDENTICAL causal block structure, so a single SPMD program serves all
cores; every half-dependence (rope phases, diagonal masks, embedding rows)
is per-core input data.  Per layer the tied-QKV tensor w (post rope +
qk-norm, bf16) is exchanged inside each pair with an AllGather.

v2 structure (vs v1):
- Attention computed in transposed-score form: sT[k,q] = wT_kblk.T @ wT_q,
  exp without max subtraction (|s*scale| <= sqrt(128), safe in f32), and
  softmax normalization deferred: se[q] accumulated by a ones-column
  matmul over p~T, applied to the attention output via a broadcast matmul
  (rsT) after AV.  This kills the per-(h,lt) p-transposes and max plumbing.
- Own-rank attention (scores/exp/se/AV vs the core's own keys) runs purely
  from local tiles and is emitted BEFORE any cc_out consumer, so it
  overlaps the AllGather flight; peer-rank attention reads only the peer
  half of cc_out via a partition-id-dependent dynamic DMA offset.
- AV is causal (per key-block j only q >= j*128 columns are computed).
- cc_out is a Shared-address-space DRAM tile (faster collective path).
- Residual blend (x = lamr*x + lamx*x0) is compiled out when the inputs
  are the trivial lamr=1, lamx=0 (checked at call time; general program
  is built otherwise).
- Logits are produced in bf16 (halves output DMA); host upcasts to f32.
"""

import sys

sys.path.insert(0, "/opt/trn_rl_repo")

import numpy as np
import ml_dtypes

BF16 = ml_dtypes.bfloat16

B, T = 4, 1024
V, E, L, H = 50304, 768, 12, 6
HD = 128
HID = 3072
EPS = 1e-6
ROPE_BASE = 10000.0
SCALE = HD ** -0.5
N_CORES = 8
TQ = 512            # rows per core
NT = 4              # 128-row tiles per core
NE = 6              # 128-col tiles of E
NJC = 6             # 512-wide chunks of HID
NJT = 24            # 128-col tiles of HID
NEG = -1e10
VCH = [(s, min(512, V - s)) for s in range(0, V, 512)]   # 99 vocab chunks
import os as _os
L_RUN = int(_os.environ.get("KBENCH_LAYERS", str(L)))
VCH_RUN = int(_os.environ.get("KBENCH_VCH", str(len(VCH))))


def _rope_tables():
    ch = np.arange(0, HD, 2, dtype=np.float32)
    inv = (1.0 / (ROPE_BASE ** (ch / np.float32(HD)))).astype(np.float32)
    t = np.arange(T, dtype=np.float32)
    fr = np.outer(t, inv).astype(np.float32)
    return np.cos(fr).astype(np.float32), np.sin(fr).astype(np.float32)


def _own_rows(half):
    return 2 * np.arange(TQ) + half


def _f32(a):
    return np.asarray(a, dtype=np.float32)


def _bf(a):
    return np.asarray(a).astype(BF16)


def _bfr(a):
    return np.asarray(a).astype(BF16).astype(np.float32)


def _diag_masks_T(half):
    """dmaskT[s][ki,qi]: s=0 own rank (r=half), s=1 peer rank (r=1-half).
    0 where (2qi+half) >= (2ki+r) else NEG."""
    qi = np.arange(128)[None, :]
    ki = np.arange(128)[:, None]
    out = np.empty((2, 128, 128), dtype=np.float32)
    for s, r in ((0, half), (1, 1 - half)):
        out[s] = np.where(2 * qi + half >= 2 * ki + r, 0.0, NEG)
    return out


def _host_prep(inputs):
    idx = np.asarray(inputs["idx"])
    wte = _f32(inputs["wte"])
    prep = {}
    prep["qkvT"] = np.ascontiguousarray(
        _f32(inputs["qkv_w"]).transpose(0, 2, 1)).astype(BF16)     # [L, E, E] (e, f)
    prep["cprojT"] = np.ascontiguousarray(
        _f32(inputs["cproj_w"]).transpose(0, 2, 1)).astype(BF16)   # [L, E, E] (e, e')
    prep["dencT"] = np.ascontiguousarray(
        _f32(inputs["denc_w"]).transpose(0, 2, 1)).astype(BF16)    # [L, E, HID]
    prep["ddecT"] = np.ascontiguousarray(
        _f32(inputs["ddec_w"]).transpose(0, 2, 1)).astype(BF16)    # [L, HID, E]
    prep["lmT"] = np.ascontiguousarray(_f32(inputs["lm_head_w"]).T).astype(BF16)
    thr = _f32(inputs["thr"])
    prep["thrneg"] = np.ascontiguousarray(
        (-thr).reshape(L, NJT, 128).transpose(2, 0, 1)).astype(np.float32)
    prep["lamr"] = np.ascontiguousarray(
        np.broadcast_to(_f32(inputs["resid_lambdas"]), (128, L))).astype(np.float32)
    prep["lamx"] = np.ascontiguousarray(
        np.broadcast_to(_f32(inputs["x0_lambdas"]), (128, L))).astype(np.float32)
    prep["trivial_blend"] = bool(
        np.all(_f32(inputs["resid_lambdas"]) == 1.0)
        and np.all(_f32(inputs["x0_lambdas"]) == 0.0))

    cos, sin = _rope_tables()          # [T, 64]
    per_core = []
    for c in range(N_CORES):
        b, half = c // 2, c % 2
        rows = _own_rows(half)
        pc = {}
        pc["xemb"] = np.ascontiguousarray(wte[idx[b][rows]]).astype(np.float32)
        pc["cosr"] = np.ascontiguousarray(np.tile(cos[rows], (1, H))).astype(np.float32)
        pc["sinr"] = np.ascontiguousarray(np.tile(sin[rows], (1, H))).astype(np.float32)
        pc["dmaskT"] = _diag_masks_T(half)
        per_core.append(pc)
    return prep, per_core


# --------------------------------------------------------------------------
# numpy mirror of the exact device dataflow (bf16 casts in the same places)
# --------------------------------------------------------------------------

def _mirror_pair(prep, pcs):
    xs = []
    for half in range(2):
        xe = pcs[half]["xemb"]
        r = 1.0 / np.sqrt((xe * xe).sum(-1, keepdims=True) / E + EPS)
        xs.append((xe * r).astype(np.float32))
    x0s = [x.copy() for x in xs]

    for i in range(L_RUN):
        rl = prep["lamr"][0, i]
        xl = prep["lamx"][0, i]
        w_bfs = []
        for half in range(2):
            x = (xs[half] * rl + x0s[half] * xl).astype(np.float32)
            xs[half] = x
            r = 1.0 / np.sqrt((x * x).sum(-1, keepdims=True) / E + EPS)
            h_bf = _bfr(x * r)
            w_raw = h_bf @ _bfr(prep["qkvT"][i])          # [TQ, E]
            wh = w_raw.reshape(TQ, H, HD)
            rw = 1.0 / np.sqrt((wh * wh).sum(-1, keepdims=True) / HD + EPS)
            cosr = pcs[half]["cosr"].reshape(TQ, H, 64)
            sinr = pcs[half]["sinr"].reshape(TQ, H, 64)
            x1, x2 = wh[..., :64], wh[..., 64:]
            wn = np.concatenate(
                [x1 * cosr + x2 * sinr, x2 * cosr - x1 * sinr], axis=-1)
            w_bfs.append(_bf((wn * rw).reshape(TQ, E)))

        new_xs = []
        for half in range(2):
            x = xs[half]
            dmaskT = pcs[half]["dmaskT"]
            w_own = w_bfs[half].astype(np.float32)        # [TQ, E]
            w_peer = w_bfs[1 - half].astype(np.float32)
            o_heads = []
            for h in range(H):
                wo = w_own[:, h * 128:(h + 1) * 128]      # [k, d]
                wp = w_peer[:, h * 128:(h + 1) * 128]
                se = np.zeros((TQ,), dtype=np.float32)
                o_acc = {}
                for s, wk in ((0, wo), (1, wp)):
                    pts = []
                    for j in range(NT):
                        q0 = j * 128
                        sT = wk[q0:q0 + 128] @ w_own[q0:, h * 128:(h + 1) * 128].T
                        sT = sT.astype(np.float32)
                        sT[:, 0:128] += dmaskT[s]
                        pt = _bfr(np.exp(sT * SCALE))     # [128, W]
                        pts.append(pt)
                        se[q0:] += pt.sum(0)
                    oT = np.zeros((128, TQ), dtype=np.float32)
                    for j in range(NT):
                        q0 = j * 128
                        oT[:, q0:] += wk[q0:q0 + 128].T @ pts[j]
                    o_acc[s] = oT
                own_bf = _bfr(o_acc[0])
                rs = (1.0 / se)[None, :]
                o_heads.append(_bfr((own_bf + o_acc[1]) * rs))   # [d, q]
            o = np.concatenate([oh.T for oh in o_heads], axis=1)  # [q, E]
            x = x + _bf(o).astype(np.float32) @ _bfr(prep["cprojT"][i])
            r2 = 1.0 / np.sqrt((x * x).sum(-1, keepdims=True) / E + EPS)
            h2 = _bfr(x * r2)
            a_raw = h2 @ _bfr(prep["dencT"][i])
            thr_i = -prep["thrneg"][:, i, :].T.reshape(HID)
            aT = _bfr(np.maximum(a_raw - thr_i, 0.0))
            x = x + aT @ _bfr(prep["ddecT"][i])
            new_xs.append(x.astype(np.float32))
        xs = new_xs

    outs = []
    for half in range(2):
        x = xs[half]
        r = 1.0 / np.sqrt((x * x).sum(-1, keepdims=True) / E + EPS)
        outs.append(_bfr(_bfr(x * r) @ _bfr(prep["lmT"])))
    return outs


def kernel_numpy(**inputs):
    prep, per_core = _host_prep(inputs)
    out = np.empty((B, T, V), dtype=np.float32)
    for b in range(B):
        logits = _mirror_pair(prep, per_core[2 * b:2 * b + 2])
        for half in range(2):
            out[b, _own_rows(half)] = logits[half]
    return out


# --------------------------------------------------------------------------
# Bass/Tile kernel
# --------------------------------------------------------------------------

_NC_CACHE = {}
LAST_RESULT = None


def _build_nc(trivial_blend, n_cores=N_CORES):
    import concourse.bacc as bacc
    import concourse.mybir as mybir
    import concourse.tile as tile
    import concourse.bass as bass
    from concourse.masks import make_identity

    f32 = mybir.dt.float32
    bf16 = mybir.dt.bfloat16
    AF = mybir.ActivationFunctionType
    ALU = mybir.AluOpType

    nc = bacc.Bacc("TRN2", target_bir_lowering=False, debug=False,
                   num_devices=n_cores)

    d_xemb = nc.dram_tensor("xemb", [TQ, E], f32, kind="ExternalInput")
    d_cosr = nc.dram_tensor("cosr", [TQ, H * 64], f32, kind="ExternalInput")
    d_sinr = nc.dram_tensor("sinr", [TQ, H * 64], f32, kind="ExternalInput")
    d_dmaskT = nc.dram_tensor("dmaskT", [2, 128, 128], f32, kind="ExternalInput")
    d_qkvT = nc.dram_tensor("qkvT", [L, E, E], bf16, kind="ExternalInput")
    d_cprojT = nc.dram_tensor("cprojT", [L, E, E], bf16, kind="ExternalInput")
    d_dencT = nc.dram_tensor("dencT", [L, E, HID], bf16, kind="ExternalInput")
    d_ddecT = nc.dram_tensor("ddecT", [L, HID, E], bf16, kind="ExternalInput")
    d_lmT = nc.dram_tensor("lmT", [E, V], bf16, kind="ExternalInput")
    d_thrneg = nc.dram_tensor("thrneg", [128, L, NJT], f32, kind="ExternalInput")
    d_lamr = nc.dram_tensor("lamr", [128, L], f32, kind="ExternalInput")
    d_lamx = nc.dram_tensor("lamx", [128, L], f32, kind="ExternalInput")
    d_logits = nc.dram_tensor("logits", [TQ, V], bf16, kind="ExternalOutput")

    groups = [[2 * g, 2 * g + 1] for g in range(n_cores // 2)]

    from contextlib import ExitStack

    with tile.TileContext(nc) as tc, ExitStack() as es:
        if True:
            st = es.enter_context(tc.tile_pool(name="state", bufs=1))
            dpool = es.enter_context(tc.tile_pool(name="dram", bufs=2, space="DRAM"))
            psA = es.enter_context(tc.tile_pool(name="psA", bufs=3, space="PSUM"))
            psB = es.enter_context(tc.tile_pool(name="psB", bufs=3, space="PSUM"))
            psT = es.enter_context(tc.tile_pool(name="psT", bufs=2, space="PSUM"))
            # ---------------- persistent state ----------------
            ident = st.tile([128, 128], f32, tag="ident")
            make_identity(nc, ident[:])
            ident_bf = st.tile([128, 128], bf16, tag="ident_bf")
            nc.vector.tensor_copy(ident_bf[:], ident[:])
            # ones_col: column 0 all ones (se sum); ones_row: row 0 all ones
            ones_col = st.tile([128, 128], bf16, tag="ones_col")
            nc.vector.memset(ones_col[:], 0.0)
            nc.vector.memset(ones_col[:, 0:1], 1.0)
            ones_row = st.tile([128, 128], bf16, tag="ones_row")
            nc.vector.memset(ones_row[:], 0.0)
            nc.vector.memset(ones_row[0:1, :], 1.0)

            dmaskT_t = st.tile([128, 2, 128], f32, tag="dmaskT")
            nc.sync.dma_start(dmaskT_t[:], d_dmaskT[:].rearrange("s k q -> k s q"))

            eps_t = st.tile([128, 1], f32, tag="eps")
            nc.vector.memset(eps_t[:], EPS)
            lamr_t = st.tile([128, L], f32, tag="lamr")
            lamx_t = st.tile([128, L], f32, tag="lamx")
            thrneg_t = st.tile([128, L, NJT], f32, tag="thrneg")
            nc.sync.dma_start(lamr_t[:], d_lamr[:])
            nc.sync.dma_start(lamx_t[:], d_lamx[:])
            nc.sync.dma_start(thrneg_t[:], d_thrneg[:])

            # peer row offset inside cc_out: even core -> rows [TQ:2TQ]
            pid = nc.sync.partition_id()
            peer_off = (1 - pid % 2) * TQ

            x_t = [st.tile([128, E], f32, tag=f"x{t}", name=f"x_{t}")
                   for t in range(NT)]
            if not trivial_blend:
                x0_t = [st.tile([128, E], f32, tag=f"x0{t}", name=f"x0_{t}")
                        for t in range(NT)]
            cosr_t = [st.tile([128, H * 64], f32, tag=f"cos{t}", name=f"cosr_{t}")
                      for t in range(NT)]
            sinr_t = [st.tile([128, H * 64], f32, tag=f"sin{t}", name=f"sinr_{t}")
                      for t in range(NT)]
            for t in range(NT):
                nc.sync.dma_start(cosr_t[t][:], d_cosr[t * 128:(t + 1) * 128, :])
                nc.sync.dma_start(sinr_t[t][:], d_sinr[t * 128:(t + 1) * 128, :])
            hfT = [st.tile([128, TQ], bf16, tag=f"hfT{e}", name=f"hfT_{e}")
                   for e in range(NE)]

            def rmsnorm_stats(src_tiles, dim, sm, junk_pool):
                rs = []
                for t in range(NT):
                    junk = junk_pool.tile([128, dim], f32, tag="sc")
                    ssq = sm.tile([128, 1], f32, tag="nss")
                    nc.scalar.activation(junk[:], src_tiles[t][:], AF.Square,
                                         accum_out=ssq[:])
                    sq = sm.tile([128, 1], f32, tag="nsq")
                    nc.scalar.activation(sq[:], ssq[:], AF.Sqrt,
                                         bias=eps_t[:], scale=1.0 / dim)
                    r = sm.tile([128, 1], f32, tag="nr")
                    nc.vector.reciprocal(r[:], sq[:])
                    rs.append(r)
                return rs

            with ExitStack() as les:
                wq_p = les.enter_context(tc.tile_pool(name="wq", bufs=2 * NE))
                wc_p = les.enter_context(tc.tile_pool(name="wc", bufs=2 * NE))
                wd_p = les.enter_context(tc.tile_pool(name="wd", bufs=12))
                wdd_p = les.enter_context(tc.tile_pool(name="wdd", bufs=8))
                sc_p = les.enter_context(tc.tile_pool(name="sc", bufs=5))
                rp_p = les.enter_context(tc.tile_pool(name="rp", bufs=2))
                hh_p = les.enter_context(tc.tile_pool(name="hh", bufs=4))
                ht_p = les.enter_context(tc.tile_pool(name="ht", bufs=7))
                sm_p = les.enter_context(tc.tile_pool(name="sm", bufs=12))
                se_p = les.enter_context(tc.tile_pool(name="se", bufs=7))
                wb_p = les.enter_context(tc.tile_pool(name="wb", bufs=4))
                wo_p = les.enter_context(tc.tile_pool(name="wo", bufs=6))
                wl_p = les.enter_context(tc.tile_pool(name="wl", bufs=3))
                wt_p = les.enter_context(tc.tile_pool(name="wt", bufs=3))
                pp_p = les.enter_context(tc.tile_pool(name="pp", bufs=5))
                ot_p = les.enter_context(tc.tile_pool(name="ot", bufs=13))
                at_p = les.enter_context(tc.tile_pool(name="at", bufs=6))
                # ---------------- embedding + initial rmsnorm ----------------
                xe_tiles = []
                for t in range(NT):
                    xe = sc_p.tile([128, E], f32, tag="sc")
                    nc.sync.dma_start(xe[:], d_xemb[t * 128:(t + 1) * 128, :])
                    xe_tiles.append(xe)
                r_emb = rmsnorm_stats(xe_tiles, E, sm_p, sc_p)
                for t in range(NT):
                    nc.scalar.activation(x_t[t][:], xe_tiles[t][:], AF.Copy,
                                         scale=r_emb[t][:])
                    if not trivial_blend:
                        nc.vector.tensor_copy(x0_t[t][:], x_t[t][:])

                # ---------------- layers ----------------
                for i in range(L_RUN):
                    qkvT_sb, cprojT_sb = [], []
                    for e in range(NE):
                        wtile = wq_p.tile([128, E], bf16, tag="wq")
                        nc.sync.dma_start(
                            wtile[:], d_qkvT[i, e * 128:(e + 1) * 128, :])
                        qkvT_sb.append(wtile)
                        ctile = wc_p.tile([128, E], bf16, tag="wc")
                        nc.sync.dma_start(
                            ctile[:], d_cprojT[i, e * 128:(e + 1) * 128, :])
                        cprojT_sb.append(ctile)

                    # residual blend: x = lamr*x + lamx*x0 (identity when trivial)
                    if not trivial_blend:
                        for t in range(NT):
                            tmp = sc_p.tile([128, E], f32, tag="sc")
                            nc.scalar.activation(tmp[:], x0_t[t][:], AF.Copy,
                                                 scale=lamx_t[:, i:i + 1])
                            nc.vector.scalar_tensor_tensor(
                                out=x_t[t][:], in0=x_t[t][:],
                                scalar=lamr_t[:, i:i + 1], in1=tmp[:],
                                op0=ALU.mult, op1=ALU.add)

                    # ---- attention input norm -> h (bf16) -> hT ----
                    r_at = rmsnorm_stats(x_t, E, sm_p, sc_p)
                    h_tiles = []
                    for t in range(NT):
                        h = hh_p.tile([128, E], bf16, tag="hh")
                        nc.scalar.activation(h[:], x_t[t][:], AF.Copy,
                                             scale=r_at[t][:])
                        h_tiles.append(h)
                    hT = []
                    for e in range(NE):
                        tp = psT.tile([128, TQ], bf16, tag="psT")
                        for t in range(NT):
                            nc.tensor.transpose(
                                tp[:, t * 128:(t + 1) * 128],
                                h_tiles[t][:, e * 128:(e + 1) * 128],
                                ident_bf[:])
                        hsb = ht_p.tile([128, TQ], bf16, tag="ht")
                        nc.vector.tensor_copy(hsb[:], tp[:])
                        hT.append(hsb)

                    # ---- qkv matmul + qk-norm + rope -> w_bf; ship to AG ----
                    cc_in = dpool.tile([TQ, E], bf16, tag="cc_in")
                    cc_out = dpool.tile([2 * TQ, E], bf16, tag="cc_out",
                                        addr_space="Shared")
                    w_bf_tiles = []
                    for t in range(NT):
                        wps = [psB.tile([128, 512], f32, tag="psB", name="wps")
                               for _ in range(2)]
                        for ch in range(2):
                            for e in range(NE):
                                nc.tensor.matmul(
                                    wps[ch][:, 0:384],
                                    hT[e][:, t * 128:(t + 1) * 128],
                                    qkvT_sb[e][:, ch * 384:(ch + 1) * 384],
                                    start=(e == 0), stop=(e == NE - 1))
                        # qk-norm stats on pre-rope w (rope is a rotation)
                        ssw = sm_p.tile([128, H], f32, tag="ssw")
                        for ch in range(2):
                            for hh in range(3):
                                hgl = ch * 3 + hh
                                junk = rp_p.tile([128, 128], f32, tag="sqj")
                                nc.scalar.activation(
                                    junk[:], wps[ch][:, hh * 128:(hh + 1) * 128],
                                    AF.Square,
                                    accum_out=ssw[:, hgl:hgl + 1])
                        sqw = sm_p.tile([128, H], f32, tag="sqw")
                        nc.scalar.activation(sqw[:], ssw[:], AF.Sqrt,
                                             bias=eps_t[:], scale=1.0 / HD)
                        rw = sm_p.tile([128, H], f32, tag="rw")
                        nc.vector.reciprocal(rw[:], sqw[:])
                        w_bf = wb_p.tile([128, E], bf16, tag="wb")
                        for ch in range(2):
                            wv = wps[ch][:, 0:384].rearrange(
                                "p (h d) -> p h d", d=128)
                            x1 = wv[:, :, 0:64]
                            x2 = wv[:, :, 64:128]
                            cg = cosr_t[t][:, ch * 192:(ch + 1) * 192] \
                                .rearrange("p (h d) -> p h d", d=64)
                            sg = sinr_t[t][:, ch * 192:(ch + 1) * 192] \
                                .rearrange("p (h d) -> p h d", d=64)
                            t1 = rp_p.tile([128, 192], f32, tag="r1")
                            t2 = rp_p.tile([128, 192], f32, tag="r2")
                            t3 = rp_p.tile([128, 192], f32, tag="r3")
                            t4 = rp_p.tile([128, 192], f32, tag="r4")
                            v1 = t1[:].rearrange("p (h d) -> p h d", d=64)
                            v2 = t2[:].rearrange("p (h d) -> p h d", d=64)
                            v3 = t3[:].rearrange("p (h d) -> p h d", d=64)
                            v4 = t4[:].rearrange("p (h d) -> p h d", d=64)
                            nc.vector.tensor_mul(v1, x1, cg)
                            nc.vector.tensor_mul(v2, x2, sg)
                            nc.vector.tensor_mul(v3, x2, cg)
                            nc.vector.tensor_mul(v4, x1, sg)
                            wn = rp_p.tile([128, 384], f32, tag="wn")
                            wnv = wn[:].rearrange("p (h d) -> p h d", d=128)
                            nc.vector.tensor_add(wnv[:, :, 0:64], v1, v2)
                            nc.vector.tensor_sub(wnv[:, :, 64:128], v3, v4)
                            for hh in range(3):
                                hgl = ch * 3 + hh
                                nc.scalar.activation(
                                    w_bf[:, hgl * 128:(hgl + 1) * 128],
                                    wn[:, hh * 128:(hh + 1) * 128],
                                    AF.Copy, scale=rw[:, hgl:hgl + 1])
                        nc.sync.dma_start(
                            cc_in[t * 128:(t + 1) * 128, :], w_bf[:])
                        w_bf_tiles.append(w_bf)

                    nc.gpsimd.collective_compute(
                        "AllGather", mybir.AluOpType.bypass,
                        replica_groups=groups,
                        ins=[cc_in[:]], outs=[cc_out[:]])

                    # own queries/keys, transposed per head: wTown[h] = [d, q]
                    wTown = []
                    for h in range(H):
                        tp = psT.tile([128, TQ], bf16, tag="psT")
                        for t in range(NT):
                            nc.tensor.transpose(
                                tp[:, t * 128:(t + 1) * 128],
                                w_bf_tiles[t][:, h * 128:(h + 1) * 128],
                                ident_bf[:])
                        wsb = wo_p.tile([128, TQ], bf16, tag="wo")
                        nc.vector.tensor_copy(wsb[:], tp[:])
                        wTown.append(wsb)

                    # ---- OWN phase: scores/exp/se/AV vs local keys ----
                    own_o, own_se = [], []
                    for h in range(H):
                        pts = []
                        for j in range(NT):
                            W = TQ - j * 128
                            sps = psA.tile([128, 512], f32, tag="psA",
                                           name="sps")
                            nc.tensor.matmul(
                                sps[:, 0:W],
                                wTown[h][:, j * 128:(j + 1) * 128],
                                wTown[h][:, j * 128:TQ],
                                start=True, stop=True)
                            nc.vector.tensor_add(
                                sps[:, 0:128], sps[:, 0:128],
                                dmaskT_t[:, 0, :])
                            pt = pp_p.tile([128, 512], bf16, tag="pp")
                            nc.scalar.activation(pt[:, 0:W], sps[:, 0:W],
                                                 AF.Exp, scale=SCALE)
                            pts.append(pt)
                        sep = psA.tile([128, 512], f32, tag="psA", name="sep")
                        for j in range(NT):
                            W = TQ - j * 128
                            nc.tensor.matmul(
                                sep[:, j * 128:TQ], ones_col[:],
                                pts[j][:, 0:W],
                                start=(j == 0), stop=(j == NT - 1))
                        se_sb = se_p.tile([128, 512], f32, tag="sesb")
                        nc.vector.tensor_copy(se_sb[:], sep[:])
                        own_se.append(se_sb)
                        avp = psB.tile([128, 512], f32, tag="psB", name="avp")
                        for j in range(NT):
                            W = TQ - j * 128
                            nc.tensor.matmul(
                                avp[:, j * 128:TQ],
                                w_bf_tiles[j][:, h * 128:(h + 1) * 128],
                                pts[j][:, 0:W],
                                start=(j == 0), stop=(j == NT - 1))
                        o_sb = ot_p.tile([128, 512], bf16, tag="ot")
                        nc.vector.tensor_copy(o_sb[:], avp[:])
                        own_o.append(o_sb)

                    # ---- PEER phase: after AllGather ----
                    oT_final = []
                    for h in range(H):
                        wallp = wl_p.tile([128, 4, 128], bf16, tag="wl")
                        nc.sync.dma_start(
                            wallp[:],
                            cc_out[bass.ds(peer_off, TQ),
                                   h * 128:(h + 1) * 128]
                            .rearrange("(n p) d -> p n d", p=128))
                        wtp = psT.tile([128, 4, 128], bf16, tag="psT")
                        for j in range(NT):
                            nc.tensor.transpose(
                                wtp[:, j, :], wallp[:, j, :], ident_bf[:])
                        wTpeer = wt_p.tile([128, TQ], bf16, tag="wt")
                        nc.vector.tensor_copy(
                            wTpeer[:], wtp[:].rearrange("p n d -> p (n d)"))
                        pts = []
                        for j in range(NT):
                            W = TQ - j * 128
                            sps = psA.tile([128, 512], f32, tag="psA",
                                           name="sps")
                            nc.tensor.matmul(
                                sps[:, 0:W],
                                wTpeer[:, j * 128:(j + 1) * 128],
                                wTown[h][:, j * 128:TQ],
                                start=True, stop=True)
                            nc.vector.tensor_add(
                                sps[:, 0:128], sps[:, 0:128],
                                dmaskT_t[:, 1, :])
                            pt = pp_p.tile([128, 512], bf16, tag="pp")
                            nc.scalar.activation(pt[:, 0:W], sps[:, 0:W],
                                                 AF.Exp, scale=SCALE)
                            pts.append(pt)
                        sep = psA.tile([128, 512], f32, tag="psA", name="sep")
                        for j in range(NT):
                            W = TQ - j * 128
                            nc.tensor.matmul(
                                sep[:, j * 128:TQ], ones_col[:],
                                pts[j][:, 0:W],
                                start=(j == 0), stop=(j == NT - 1))
                        se_tot = se_p.tile([128, 512], f32, tag="setot")
                        nc.vector.tensor_add(se_tot[:], own_se[h][:], sep[:])
                        bc = psB.tile([128, 512], f32, tag="psB", name="bc")
                        nc.tensor.matmul(bc[:], ones_row[:], se_tot[:],
                                         start=True, stop=True)
                        rsT = se_p.tile([128, 512], f32, tag="rsT")
                        nc.vector.reciprocal(rsT[:], bc[:])
                        avp = psB.tile([128, 512], f32, tag="psB", name="avp")
                        for j in range(NT):
                            W = TQ - j * 128
                            nc.tensor.matmul(
                                avp[:, j * 128:TQ],
                                wallp[:, j, :],
                                pts[j][:, 0:W],
                                start=(j == 0), stop=(j == NT - 1))
                        tmp = se_p.tile([128, 512], f32, tag="tmpo")
                        nc.vector.tensor_add(tmp[:], own_o[h][:], avp[:])
                        osb = ot_p.tile([128, 512], bf16, tag="ot")
                        nc.vector.tensor_mul(osb[:], tmp[:], rsT[:])
                        oT_final.append(osb)

                    # ---- cproj + residual add ----
                    for t in range(NT):
                        for ch in range(2):
                            cps = psB.tile([128, 512], f32, tag="psB",
                                           name="cps")
                            for e in range(NE):
                                nc.tensor.matmul(
                                    cps[:, 0:384],
                                    oT_final[e][:, t * 128:(t + 1) * 128],
                                    cprojT_sb[e][:, ch * 384:(ch + 1) * 384],
                                    start=(e == 0), stop=(e == NE - 1))
                            nc.vector.tensor_add(
                                x_t[t][:, ch * 384:(ch + 1) * 384],
                                x_t[t][:, ch * 384:(ch + 1) * 384],
                                cps[:, 0:384])

                    # ---- ODL ----
                    r_od = rmsnorm_stats(x_t, E, sm_p, sc_p)
                    h2_tiles = []
                    for t in range(NT):
                        h2 = hh_p.tile([128, E], bf16, tag="hh")
                        nc.scalar.activation(h2[:], x_t[t][:], AF.Copy,
                                             scale=r_od[t][:])
                        h2_tiles.append(h2)
                    h2T = []
                    for e in range(NE):
                        tp = psT.tile([128, TQ], bf16, tag="psT")
                        for t in range(NT):
                            nc.tensor.transpose(
                                tp[:, t * 128:(t + 1) * 128],
                                h2_tiles[t][:, e * 128:(e + 1) * 128],
                                ident_bf[:])
                        hsb = ht_p.tile([128, TQ], bf16, tag="ht")
                        nc.vector.tensor_copy(hsb[:], tp[:])
                        h2T.append(hsb)

                    for jc in range(NJC):
                        dtiles = []
                        for e in range(NE):
                            dt_ = wd_p.tile([128, 512], bf16, tag="wd")
                            nc.sync.dma_start(
                                dt_[:],
                                d_dencT[i, e * 128:(e + 1) * 128,
                                        jc * 512:(jc + 1) * 512])
                            dtiles.append(dt_)
                        ddtiles = []
                        for jt in range(4):
                            ddt = wdd_p.tile([128, E], bf16, tag="wdd")
                            nc.sync.dma_start(
                                ddt[:],
                                d_ddecT[i, jc * 512 + jt * 128:
                                        jc * 512 + (jt + 1) * 128, :])
                            ddtiles.append(ddt)
                        aT = []
                        for jt in range(4):
                            aps = psA.tile([128, 512], f32, tag="psA",
                                           name="aps")
                            for e in range(NE):
                                nc.tensor.matmul(
                                    aps[:],
                                    dtiles[e][:, jt * 128:(jt + 1) * 128],
                                    h2T[e][:],
                                    start=(e == 0), stop=(e == NE - 1))
                            asb = at_p.tile([128, TQ], bf16, tag="at")
                            jgl = jc * 4 + jt
                            nc.scalar.activation(
                                asb[:], aps[:], AF.Relu,
                                bias=thrneg_t[:, i, jgl:jgl + 1])
                            aT.append(asb)
                        for t in range(NT):
                            for ch in range(2):
                                dps = psB.tile([128, 512], f32, tag="psB",
                                               name="dps")
                                for jt in range(4):
                                    nc.tensor.matmul(
                                        dps[:, 0:384],
                                        aT[jt][:, t * 128:(t + 1) * 128],
                                        ddtiles[jt][:, ch * 384:(ch + 1) * 384],
                                        start=(jt == 0), stop=(jt == 3))
                                nc.vector.tensor_add(
                                    x_t[t][:, ch * 384:(ch + 1) * 384],
                                    x_t[t][:, ch * 384:(ch + 1) * 384],
                                    dps[:, 0:384])

                # ---------------- final rmsnorm -> hfT ----------------
                r_f = rmsnorm_stats(x_t, E, sm_p, sc_p)
                hf_tiles = []
                for t in range(NT):
                    hf = hh_p.tile([128, E], bf16, tag="hh")
                    nc.scalar.activation(hf[:], x_t[t][:], AF.Copy,
                                         scale=r_f[t][:])
                    hf_tiles.append(hf)
                for e in range(NE):
                    tp = psT.tile([128, TQ], bf16, tag="psT")
                    for t in range(NT):
                        nc.tensor.transpose(
                            tp[:, t * 128:(t + 1) * 128],
                            hf_tiles[t][:, e * 128:(e + 1) * 128],
                            ident_bf[:])
                    nc.vector.tensor_copy(hfT[e][:], tp[:])

            # ---------------- lm head ----------------
            with ExitStack() as mes:
                lmw_p = mes.enter_context(tc.tile_pool(name="lmw", bufs=18))
                lg_p = mes.enter_context(tc.tile_pool(name="lg", bufs=4))
                for vc, (vs, vw) in enumerate(VCH[:VCH_RUN]):
                    ltiles = []
                    for e in range(NE):
                        lw = lmw_p.tile([128, 512], bf16, tag="lmw")
                        nc.sync.dma_start(
                            lw[:, 0:vw],
                            d_lmT[e * 128:(e + 1) * 128, vs:vs + vw])
                        ltiles.append(lw)
                    for t in range(NT):
                        lps = psA.tile([128, 512], f32, tag="psA", name="lps")
                        for e in range(NE):
                            nc.tensor.matmul(
                                lps[:, 0:vw],
                                hfT[e][:, t * 128:(t + 1) * 128],
                                ltiles[e][:, 0:vw],
                                start=(e == 0), stop=(e == NE - 1))
                        lg = lg_p.tile([128, 512], bf16, tag="lg")
                        nc.vector.tensor_copy(lg[:, 0:vw], lps[:, 0:vw])
                        nc.sync.dma_start(
                            d_logits[t * 128:(t + 1) * 128, vs:vs + vw],
                            lg[:, 0:vw])

    nc.compile()
    return nc


def kernel(**inputs):
    global LAST_RESULT
    from concourse.bass_utils import run_bass_kernel_spmd

    prep, per_core = _host_prep(inputs)
    key = prep["trivial_blend"]
    if key not in _NC_CACHE:
        _NC_CACHE[key] = _build_nc(key)
    nc = _NC_CACHE[key]

    in_maps = []
    for c in range(N_CORES):
        pc = per_core[c]
        in_maps.append({
            "xemb": pc["xemb"], "cosr": pc["cosr"], "sinr": pc["sinr"],
            "dmaskT": pc["dmaskT"],
            "qkvT": prep["qkvT"], "cprojT": prep["cprojT"],
            "dencT": prep["dencT"], "ddecT": prep["ddecT"],
            "lmT": prep["lmT"], "thrneg": prep["thrneg"],
            "lamr": prep["lamr"], "lamx": prep["lamx"],
        })
    trace = bool(_os.environ.get("KBENCH_TRACE"))
    res = run_bass_kernel_spmd(nc, in_maps, core_ids=list(range(N_CORES)),
                               trace=trace,
                               trace_cores=list(range(N_CORES)) if trace else None)
    LAST_RESULT = res
    out = np.empty((B, T, V), dtype=np.float32)
    for c in range(N_CORES):
        b, half = c // 2, c % 2
        out[b, _own_rows(half)] = np.asarray(
            res.results[c]["logits"]).astype(np.float32)
    return out


# revision 9
# speedup vs baseline: 1.3017x; 1.3017x over previous
"""Trainium2 Bass kernel for nn_CRATE (12-layer CRATE-style transformer).

Sharding over 8 NeuronCores: 4 batch groups x 2-way parity-interleaved
sequence split.  Core c handles batch b=c//2 and parity half=c%2: it owns
absolute rows {2*j + half, j=0..511}.  With this split both halves have an
IDENTICAL causal block structure, so a single SPMD program serves all
cores; every half-dependence (rope phases, diagonal masks, embedding rows)
is per-core input data.  Per layer the tied-QKV tensor w (post rope +
qk-norm, bf16) is exchanged inside each pair with an AllGather.

v2 structure (vs v1):
- Attention computed in transposed-score form: sT[k,q] = wT_kblk.T @ wT_q,
  exp without max subtraction (|s*scale| <= sqrt(128), safe in f32), and
  softmax normalization deferred: se[q] accumulated by a ones-column
  matmul over p~T, applied to the attention output via a broadcast matmul
  (rsT) after AV.  This kills the per-(h,lt) p-transposes and max plumbing.
- Own-rank attention (scores/exp/se/AV vs the core's own keys) runs purely
  from local tiles and is emitted BEFORE any cc_out consumer, so it
  overlaps the AllGather flight; peer-rank attention reads only the peer
  half of cc_out via a partition-id-dependent dynamic DMA offset.
- AV is causal (per key-block j only q >= j*128 columns are computed).
- cc_out is a Shared-address-space DRAM tile (faster collective path).
- Residual blend (x = lamr*x + lamx*x0) is compiled out when the inputs
  are the trivial lamr=1, lamx=0 (checked at call time; general program
  is built otherwise).
- Logits are produced in bf16 (halves output DMA); host upcasts to f32.
"""

import sys

sys.path.insert(0, "/opt/trn_rl_repo")

import numpy as np
import ml_dtypes

BF16 = ml_dtypes.bfloat16

B, T = 4, 1024
V, E, L, H = 50304, 768, 12, 6
HD = 128
HID = 3072
EPS = 1e-6
ROPE_BASE = 10000.0
SCALE = HD ** -0.5
N_CORES = 8
TQ = 512            # rows per core
NT = 4              # 128-row tiles per core
NE = 6              # 128-col tiles of E
NJC = 6             # 512-wide chunks of HID
NJT = 24            # 128-col tiles of HID
NEG = -1e10
VCH = [(s, min(512, V - s)) for s in range(0, V, 512)]   # 99 vocab chunks
import os as _os
L_RUN = int(_os.environ.get("KBENCH_LAYERS", str(L)))
VCH_RUN = int(_os.environ.get("KBENCH_VCH", str(len(VCH))))


def _rope_tables():
    ch = np.arange(0, HD, 2, dtype=np.float32)
    inv = (1.0 / (ROPE_BASE ** (ch / np.float32(HD)))).astype(np.float32)
    t = np.arange(T, dtype=np.float32)
    fr = np.outer(t, inv).astype(np.float32)
    return np.cos(fr).astype(np.float32), np.sin(fr).astype(np.float32)


def _own_rows(half):
    return 2 * np.arange(TQ) + half


def _f32(a):
    return np.asarray(a, dtype=np.float32)


def _bf(a):
    return np.asarray(a).astype(BF16)


def _bfr(a):
    return np.asarray(a).astype(BF16).astype(np.float32)


def _diag_masks_T(half):
    """dmaskT[s][ki,qi]: s=0 own rank (r=half), s=1 peer rank (r=1-half).
    0 where (2qi+half) >= (2ki+r) else NEG."""
    qi = np.arange(128)[None, :]
    ki = np.arange(128)[:, None]
    out = np.empty((2, 128, 128), dtype=np.float32)
    for s, r in ((0, half), (1, 1 - half)):
        out[s] = np.where(2 * qi + half >= 2 * ki + r, 0.0, NEG)
    return out


def _host_prep(inputs):
    idx = np.asarray(inputs["idx"])
    wte = _f32(inputs["wte"])
    prep = {}
    prep["qkvT"] = np.ascontiguousarray(
        _f32(inputs["qkv_w"]).transpose(0, 2, 1)).astype(BF16)     # [L, E, E] (e, f)
    prep["cprojT"] = np.ascontiguousarray(
        _f32(inputs["cproj_w"]).transpose(0, 2, 1)).astype(BF16)   # [L, E, E] (e, e')
    prep["dencT"] = np.ascontiguousarray(
        _f32(inputs["denc_w"]).transpose(0, 2, 1)).astype(BF16)    # [L, E, HID]
    prep["ddecT"] = np.ascontiguousarray(
        _f32(inputs["ddec_w"]).transpose(0, 2, 1)).astype(BF16)    # [L, HID, E]
    prep["lmT"] = np.ascontiguousarray(_f32(inputs["lm_head_w"]).T).astype(BF16)
    thr = _f32(inputs["thr"])
    prep["thrneg"] = np.ascontiguousarray(
        (-thr).reshape(L, NJT, 128).transpose(2, 0, 1)).astype(np.float32)
    prep["lamr"] = np.ascontiguousarray(
        np.broadcast_to(_f32(inputs["resid_lambdas"]), (128, L))).astype(np.float32)
    prep["lamx"] = np.ascontiguousarray(
        np.broadcast_to(_f32(inputs["x0_lambdas"]), (128, L))).astype(np.float32)
    prep["trivial_blend"] = bool(
        np.all(_f32(inputs["resid_lambdas"]) == 1.0)
        and np.all(_f32(inputs["x0_lambdas"]) == 0.0))

    cos, sin = _rope_tables()          # [T, 64]
    per_core = []
    for c in range(N_CORES):
        b, half = c // 2, c % 2
        rows = _own_rows(half)
        pc = {}
        pc["xemb"] = np.ascontiguousarray(wte[idx[b][rows]]).astype(np.float32)
        pc["cosr"] = np.ascontiguousarray(np.tile(cos[rows], (1, H))).astype(np.float32)
        pc["sinr"] = np.ascontiguousarray(np.tile(sin[rows], (1, H))).astype(np.float32)
        pc["dmaskT"] = _diag_masks_T(half)
        per_core.append(pc)
    return prep, per_core


# --------------------------------------------------------------------------
# numpy mirror of the exact device dataflow (bf16 casts in the same places)
# --------------------------------------------------------------------------

def _mirror_pair(prep, pcs):
    xs = []
    for half in range(2):
        xe = pcs[half]["xemb"]
        r = 1.0 / np.sqrt((xe * xe).sum(-1, keepdims=True) / E + EPS)
        xs.append((xe * r).astype(np.float32))
    x0s = [x.copy() for x in xs]

    for i in range(L_RUN):
        rl = prep["lamr"][0, i]
        xl = prep["lamx"][0, i]
        w_bfs = []
        for half in range(2):
            x = (xs[half] * rl + x0s[half] * xl).astype(np.float32)
            xs[half] = x
            r = 1.0 / np.sqrt((x * x).sum(-1, keepdims=True) / E + EPS)
            h_bf = _bfr(x * r)
            w_raw = h_bf @ _bfr(prep["qkvT"][i])          # [TQ, E]
            wh = w_raw.reshape(TQ, H, HD)
            rw = 1.0 / np.sqrt((wh * wh).sum(-1, keepdims=True) / HD + EPS)
            cosr = pcs[half]["cosr"].reshape(TQ, H, 64)
            sinr = pcs[half]["sinr"].reshape(TQ, H, 64)
            x1, x2 = wh[..., :64], wh[..., 64:]
            wn = np.concatenate(
                [x1 * cosr + x2 * sinr, x2 * cosr - x1 * sinr], axis=-1)
            w_bfs.append(_bf((wn * rw).reshape(TQ, E)))

        new_xs = []
        for half in range(2):
            x = xs[half]
            dmaskT = pcs[half]["dmaskT"]
            w_own = w_bfs[half].astype(np.float32)        # [TQ, E]
            w_peer = w_bfs[1 - half].astype(np.float32)
            o_heads = []
            for h in range(H):
                wo = w_own[:, h * 128:(h + 1) * 128]      # [k, d]
                wp = w_peer[:, h * 128:(h + 1) * 128]
                se = np.zeros((TQ,), dtype=np.float32)
                o_acc = {}
                for s, wk in ((0, wo), (1, wp)):
                    pts = []
                    for j in range(NT):
                        q0 = j * 128
                        sT = wk[q0:q0 + 128] @ w_own[q0:, h * 128:(h + 1) * 128].T
                        sT = sT.astype(np.float32)
                        sT[:, 0:128] += dmaskT[s]
                        pt = _bfr(np.exp(sT * SCALE))     # [128, W]
                        pts.append(pt)
                        se[q0:] += pt.sum(0)
                    oT = np.zeros((128, TQ), dtype=np.float32)
                    for j in range(NT):
                        q0 = j * 128
                        oT[:, q0:] += wk[q0:q0 + 128].T @ pts[j]
                    o_acc[s] = oT
                own_bf = _bfr(o_acc[0])
                rs = (1.0 / _bfr(se))[None, :]
                o_heads.append(_bfr((own_bf + o_acc[1]) * rs))   # [d, q]
            o = np.concatenate([oh.T for oh in o_heads], axis=1)  # [q, E]
            x = x + _bf(o).astype(np.float32) @ _bfr(prep["cprojT"][i])
            r2 = 1.0 / np.sqrt((x * x).sum(-1, keepdims=True) / E + EPS)
            h2 = _bfr(x * r2)
            a_raw = h2 @ _bfr(prep["dencT"][i])
            thr_i = -prep["thrneg"][:, i, :].T.reshape(HID)
            aT = _bfr(np.maximum(a_raw - thr_i, 0.0))
            x = x + aT @ _bfr(prep["ddecT"][i])
            new_xs.append(x.astype(np.float32))
        xs = new_xs

    outs = []
    for half in range(2):
        x = xs[half]
        r = 1.0 / np.sqrt((x * x).sum(-1, keepdims=True) / E + EPS)
        outs.append(_bfr(_bfr(x * r) @ _bfr(prep["lmT"])))
    return outs


def kernel_numpy(**inputs):
    prep, per_core = _host_prep(inputs)
    out = np.empty((B, T, V), dtype=np.float32)
    for b in range(B):
        logits = _mirror_pair(prep, per_core[2 * b:2 * b + 2])
        for half in range(2):
            out[b, _own_rows(half)] = logits[half]
    return out


# --------------------------------------------------------------------------
# Bass/Tile kernel
# --------------------------------------------------------------------------

_NC_CACHE = {}
LAST_RESULT = None


def _build_nc(trivial_blend, n_cores=N_CORES):
    import concourse.bacc as bacc
    import concourse.mybir as mybir
    import concourse.tile as tile
    import concourse.bass as bass
    from concourse.masks import make_identity

    f32 = mybir.dt.float32
    bf16 = mybir.dt.bfloat16
    AF = mybir.ActivationFunctionType
    ALU = mybir.AluOpType

    nc = bacc.Bacc("TRN2", target_bir_lowering=False, debug=False,
                   num_devices=n_cores)

    d_xemb = nc.dram_tensor("xemb", [TQ, E], f32, kind="ExternalInput")
    d_cosr = nc.dram_tensor("cosr", [TQ, H * 64], f32, kind="ExternalInput")
    d_sinr = nc.dram_tensor("sinr", [TQ, H * 64], f32, kind="ExternalInput")
    d_dmaskT = nc.dram_tensor("dmaskT", [2, 128, 128], f32, kind="ExternalInput")
    d_qkvT = nc.dram_tensor("qkvT", [L, E, E], bf16, kind="ExternalInput")
    d_cprojT = nc.dram_tensor("cprojT", [L, E, E], bf16, kind="ExternalInput")
    d_dencT = nc.dram_tensor("dencT", [L, E, HID], bf16, kind="ExternalInput")
    d_ddecT = nc.dram_tensor("ddecT", [L, HID, E], bf16, kind="ExternalInput")
    d_lmT = nc.dram_tensor("lmT", [E, V], bf16, kind="ExternalInput")
    d_thrneg = nc.dram_tensor("thrneg", [128, L, NJT], f32, kind="ExternalInput")
    d_lamr = nc.dram_tensor("lamr", [128, L], f32, kind="ExternalInput")
    d_lamx = nc.dram_tensor("lamx", [128, L], f32, kind="ExternalInput")
    d_logits = nc.dram_tensor("logits", [TQ, V], bf16, kind="ExternalOutput")

    groups = [[2 * g, 2 * g + 1] for g in range(n_cores // 2)]

    from contextlib import ExitStack

    with tile.TileContext(nc) as tc, ExitStack() as es:
        if True:
            st = es.enter_context(tc.tile_pool(name="state", bufs=1))
            dpool = es.enter_context(tc.tile_pool(name="dram", bufs=2, space="DRAM"))
            psA = es.enter_context(tc.tile_pool(name="psA", bufs=3, space="PSUM"))
            psB = es.enter_context(tc.tile_pool(name="psB", bufs=3, space="PSUM"))
            psT = es.enter_context(tc.tile_pool(name="psT", bufs=2, space="PSUM"))
            # ---------------- persistent state ----------------
            ident = st.tile([128, 128], f32, tag="ident")
            make_identity(nc, ident[:])
            ident_bf = st.tile([128, 128], bf16, tag="ident_bf")
            nc.vector.tensor_copy(ident_bf[:], ident[:])
            # ones_col: column 0 all ones (se sum); ones_row: row 0 all ones
            ones_col = st.tile([128, 128], bf16, tag="ones_col")
            nc.vector.memset(ones_col[:], 0.0)
            nc.vector.memset(ones_col[:, 0:1], 1.0)
            ones_row = st.tile([128, 128], bf16, tag="ones_row")
            nc.vector.memset(ones_row[:], 0.0)
            nc.vector.memset(ones_row[0:1, :], 1.0)

            dmaskT_t = st.tile([128, 2, 128], f32, tag="dmaskT")
            nc.sync.dma_start(dmaskT_t[:], d_dmaskT[:].rearrange("s k q -> k s q"))

            eps_t = st.tile([128, 1], f32, tag="eps")
            nc.vector.memset(eps_t[:], EPS)
            lamr_t = st.tile([128, L], f32, tag="lamr")
            lamx_t = st.tile([128, L], f32, tag="lamx")
            thrneg_t = st.tile([128, L, NJT], f32, tag="thrneg")
            nc.sync.dma_start(lamr_t[:], d_lamr[:])
            nc.sync.dma_start(lamx_t[:], d_lamx[:])
            nc.sync.dma_start(thrneg_t[:], d_thrneg[:])

            # peer row offset inside cc_out: even core -> rows [TQ:2TQ]
            pid = nc.sync.partition_id()
            peer_off = (1 - pid % 2) * TQ

            x_t = [st.tile([128, E], f32, tag=f"x{t}", name=f"x_{t}")
                   for t in range(NT)]
            if not trivial_blend:
                x0_t = [st.tile([128, E], f32, tag=f"x0{t}", name=f"x0_{t}")
                        for t in range(NT)]
            cosr_t = [st.tile([128, H * 64], f32, tag=f"cos{t}", name=f"cosr_{t}")
                      for t in range(NT)]
            sinr_t = [st.tile([128, H * 64], f32, tag=f"sin{t}", name=f"sinr_{t}")
                      for t in range(NT)]
            for t in range(NT):
                nc.sync.dma_start(cosr_t[t][:], d_cosr[t * 128:(t + 1) * 128, :])
                nc.sync.dma_start(sinr_t[t][:], d_sinr[t * 128:(t + 1) * 128, :])
            hfT = [st.tile([128, TQ], bf16, tag=f"hfT{e}", name=f"hfT_{e}")
                   for e in range(NE)]

            def rmsnorm_stats(src_tiles, dim, sm, junk_pool):
                rs = []
                for t in range(NT):
                    junk = junk_pool.tile([128, dim], f32, tag="sc")
                    ssq = sm.tile([128, 1], f32, tag="nss")
                    nc.scalar.activation(junk[:], src_tiles[t][:], AF.Square,
                                         accum_out=ssq[:])
                    sq = sm.tile([128, 1], f32, tag="nsq")
                    nc.scalar.activation(sq[:], ssq[:], AF.Sqrt,
                                         bias=eps_t[:], scale=1.0 / dim)
                    r = sm.tile([128, 1], f32, tag="nr")
                    nc.vector.reciprocal(r[:], sq[:])
                    rs.append(r)
                return rs

            with ExitStack() as les:
                wq_p = les.enter_context(tc.tile_pool(name="wq", bufs=2 * NE))
                wc_p = les.enter_context(tc.tile_pool(name="wc", bufs=2 * NE))
                wd_p = les.enter_context(tc.tile_pool(name="wd", bufs=12))
                wdd_p = les.enter_context(tc.tile_pool(name="wdd", bufs=8))
                sc_p = les.enter_context(tc.tile_pool(name="sc", bufs=5))
                rp_p = les.enter_context(tc.tile_pool(name="rp", bufs=2))
                hh_p = les.enter_context(tc.tile_pool(name="hh", bufs=4))
                ht_p = les.enter_context(tc.tile_pool(name="ht", bufs=7))
                sm_p = les.enter_context(tc.tile_pool(name="sm", bufs=12))
                se_p = les.enter_context(tc.tile_pool(name="se", bufs=7))
                wb_p = les.enter_context(tc.tile_pool(name="wb", bufs=4))
                wo_p = les.enter_context(tc.tile_pool(name="wo", bufs=6))
                wl_p = les.enter_context(tc.tile_pool(name="wl", bufs=3))
                wt_p = les.enter_context(tc.tile_pool(name="wt", bufs=3))
                pp_p = les.enter_context(tc.tile_pool(name="pp", bufs=5))
                ot_p = les.enter_context(tc.tile_pool(name="ot", bufs=13))
                at_p = les.enter_context(tc.tile_pool(name="at", bufs=6))
                # ---------------- embedding + initial rmsnorm ----------------
                xe_tiles = []
                for t in range(NT):
                    xe = sc_p.tile([128, E], f32, tag="sc")
                    nc.sync.dma_start(xe[:], d_xemb[t * 128:(t + 1) * 128, :])
                    xe_tiles.append(xe)
                r_emb = rmsnorm_stats(xe_tiles, E, sm_p, sc_p)
                for t in range(NT):
                    nc.scalar.activation(x_t[t][:], xe_tiles[t][:], AF.Copy,
                                         scale=r_emb[t][:])
                    if not trivial_blend:
                        nc.vector.tensor_copy(x0_t[t][:], x_t[t][:])

                # ---------------- layers ----------------
                for i in range(L_RUN):
                    qkvT_sb, cprojT_sb = [], []
                    for e in range(NE):
                        wtile = wq_p.tile([128, E], bf16, tag="wq")
                        nc.sync.dma_start(
                            wtile[:], d_qkvT[i, e * 128:(e + 1) * 128, :])
                        qkvT_sb.append(wtile)
                        ctile = wc_p.tile([128, E], bf16, tag="wc")
                        nc.sync.dma_start(
                            ctile[:], d_cprojT[i, e * 128:(e + 1) * 128, :])
                        cprojT_sb.append(ctile)

                    # residual blend: x = lamr*x + lamx*x0 (identity when trivial)
                    if not trivial_blend:
                        for t in range(NT):
                            tmp = sc_p.tile([128, E], f32, tag="sc")
                            nc.scalar.activation(tmp[:], x0_t[t][:], AF.Copy,
                                                 scale=lamx_t[:, i:i + 1])
                            nc.vector.scalar_tensor_tensor(
                                out=x_t[t][:], in0=x_t[t][:],
                                scalar=lamr_t[:, i:i + 1], in1=tmp[:],
                                op0=ALU.mult, op1=ALU.add)

                    # ---- attention input norm -> h (bf16) -> hT ----
                    r_at = rmsnorm_stats(x_t, E, sm_p, sc_p)
                    h_tiles = []
                    for t in range(NT):
                        h = hh_p.tile([128, E], bf16, tag="hh")
                        nc.scalar.activation(h[:], x_t[t][:], AF.Copy,
                                             scale=r_at[t][:])
                        h_tiles.append(h)
                    hT = []
                    for e in range(NE):
                        tp = psT.tile([128, TQ], bf16, tag="psT")
                        for t in range(NT):
                            nc.tensor.transpose(
                                tp[:, t * 128:(t + 1) * 128],
                                h_tiles[t][:, e * 128:(e + 1) * 128],
                                ident_bf[:])
                        hsb = ht_p.tile([128, TQ], bf16, tag="ht")
                        nc.vector.tensor_copy(hsb[:], tp[:])
                        hT.append(hsb)

                    # ---- qkv matmul + qk-norm + rope -> w_bf; ship to AG ----
                    cc_in = dpool.tile([TQ, E], bf16, tag="cc_in")
                    cc_out = dpool.tile([2 * TQ, E], bf16, tag="cc_out")
                    w_bf_tiles = []
                    for t in range(NT):
                        wps = [psB.tile([128, 512], f32, tag="psB", name="wps")
                               for _ in range(2)]
                        for ch in range(2):
                            for e in range(NE):
                                nc.tensor.matmul(
                                    wps[ch][:, 0:384],
                                    hT[e][:, t * 128:(t + 1) * 128],
                                    qkvT_sb[e][:, ch * 384:(ch + 1) * 384],
                                    start=(e == 0), stop=(e == NE - 1))
                        # qk-norm stats on pre-rope w (rope is a rotation)
                        ssw = sm_p.tile([128, H], f32, tag="ssw")
                        for ch in range(2):
                            for hh in range(3):
                                hgl = ch * 3 + hh
                                junk = rp_p.tile([128, 128], f32, tag="sqj")
                                nc.scalar.activation(
                                    junk[:], wps[ch][:, hh * 128:(hh + 1) * 128],
                                    AF.Square,
                                    accum_out=ssw[:, hgl:hgl + 1])
                        sqw = sm_p.tile([128, H], f32, tag="sqw")
                        nc.scalar.activation(sqw[:], ssw[:], AF.Sqrt,
                                             bias=eps_t[:], scale=1.0 / HD)
                        rw = sm_p.tile([128, H], f32, tag="rw")
                        nc.vector.reciprocal(rw[:], sqw[:])
                        w_bf = wb_p.tile([128, E], bf16, tag="wb")
                        for ch in range(2):
                            wv = wps[ch][:, 0:384].rearrange(
                                "p (h d) -> p h d", d=128)
                            x1 = wv[:, :, 0:64]
                            x2 = wv[:, :, 64:128]
                            cg = cosr_t[t][:, ch * 192:(ch + 1) * 192] \
                                .rearrange("p (h d) -> p h d", d=64)
                            sg = sinr_t[t][:, ch * 192:(ch + 1) * 192] \
                                .rearrange("p (h d) -> p h d", d=64)
                            t1 = rp_p.tile([128, 192], f32, tag="r1")
                            t2 = rp_p.tile([128, 192], f32, tag="r2")
                            t3 = rp_p.tile([128, 192], f32, tag="r3")
                            t4 = rp_p.tile([128, 192], f32, tag="r4")
                            v1 = t1[:].rearrange("p (h d) -> p h d", d=64)
                            v2 = t2[:].rearrange("p (h d) -> p h d", d=64)
                            v3 = t3[:].rearrange("p (h d) -> p h d", d=64)
                            v4 = t4[:].rearrange("p (h d) -> p h d", d=64)
                            nc.vector.tensor_mul(v1, x1, cg)
                            nc.vector.tensor_mul(v2, x2, sg)
                            nc.vector.tensor_mul(v3, x2, cg)
                            nc.vector.tensor_mul(v4, x1, sg)
                            wn = rp_p.tile([128, 384], f32, tag="wn")
                            wnv = wn[:].rearrange("p (h d) -> p h d", d=128)
                            nc.vector.tensor_add(wnv[:, :, 0:64], v1, v2)
                            nc.vector.tensor_sub(wnv[:, :, 64:128], v3, v4)
                            for hh in range(3):
                                hgl = ch * 3 + hh
                                nc.scalar.activation(
                                    w_bf[:, hgl * 128:(hgl + 1) * 128],
                                    wn[:, hh * 128:(hh + 1) * 128],
                                    AF.Copy, scale=rw[:, hgl:hgl + 1])
                        nc.sync.dma_start(
                            cc_in[t * 128:(t + 1) * 128, :], w_bf[:])
                        w_bf_tiles.append(w_bf)

                    nc.gpsimd.collective_compute(
                        "AllGather", mybir.AluOpType.bypass,
                        replica_groups=groups,
                        ins=[cc_in[:]], outs=[cc_out[:]])

                    # own queries/keys, transposed per head: wTown[h] = [d, q]
                    wTown = []
                    for h in range(H):
                        tp = psT.tile([128, TQ], bf16, tag="psT")
                        for t in range(NT):
                            nc.tensor.transpose(
                                tp[:, t * 128:(t + 1) * 128],
                                w_bf_tiles[t][:, h * 128:(h + 1) * 128],
                                ident_bf[:])
                        wsb = wo_p.tile([128, TQ], bf16, tag="wo")
                        nc.vector.tensor_copy(wsb[:], tp[:])
                        wTown.append(wsb)

                    # ---- OWN phase: scores/exp/se/AV vs local keys ----
                    own_o, own_se = [], []
                    for h in range(H):
                        pts = []
                        for j in range(NT):
                            W = TQ - j * 128
                            sps = psA.tile([128, 512], f32, tag="psA",
                                           name="sps")
                            nc.tensor.matmul(
                                sps[:, 0:W],
                                wTown[h][:, j * 128:(j + 1) * 128],
                                wTown[h][:, j * 128:TQ],
                                start=True, stop=True)
                            nc.vector.tensor_add(
                                sps[:, 0:128], sps[:, 0:128],
                                dmaskT_t[:, 0, :])
                            pt = pp_p.tile([128, 512], bf16, tag="pp")
                            nc.scalar.activation(pt[:, 0:W], sps[:, 0:W],
                                                 AF.Exp, scale=SCALE)
                            pts.append(pt)
                        sep = psA.tile([128, 512], f32, tag="psA", name="sep")
                        for j in range(NT):
                            W = TQ - j * 128
                            nc.tensor.matmul(
                                sep[:, j * 128:TQ], ones_col[:],
                                pts[j][:, 0:W],
                                start=(j == 0), stop=(j == NT - 1))
                        se_sb = se_p.tile([128, 512], f32, tag="sesb")
                        nc.vector.tensor_copy(se_sb[:], sep[:])
                        own_se.append(se_sb)
                        avp = psB.tile([128, 512], f32, tag="psB", name="avp")
                        for j in range(NT):
                            W = TQ - j * 128
                            nc.tensor.matmul(
                                avp[:, j * 128:TQ],
                                w_bf_tiles[j][:, h * 128:(h + 1) * 128],
                                pts[j][:, 0:W],
                                start=(j == 0), stop=(j == NT - 1))
                        o_sb = ot_p.tile([128, 512], bf16, tag="ot")
                        nc.vector.tensor_copy(o_sb[:], avp[:])
                        own_o.append(o_sb)

                    # ---- PEER phase: after AllGather ----
                    oT_final = []
                    for h in range(H):
                        wallp = wl_p.tile([128, 4, 128], bf16, tag="wl")
                        nc.sync.dma_start(
                            wallp[:],
                            cc_out[bass.ds(peer_off, TQ),
                                   h * 128:(h + 1) * 128]
                            .rearrange("(n p) d -> p n d", p=128))
                        wtp = psT.tile([128, 4, 128], bf16, tag="psT")
                        for j in range(NT):
                            nc.tensor.transpose(
                                wtp[:, j, :], wallp[:, j, :], ident_bf[:])
                        wTpeer = wt_p.tile([128, TQ], bf16, tag="wt")
                        nc.vector.tensor_copy(
                            wTpeer[:], wtp[:].rearrange("p n d -> p (n d)"))
                        pts = []
                        for j in range(NT):
                            W = TQ - j * 128
                            sps = psA.tile([128, 512], f32, tag="psA",
                                           name="sps")
                            nc.tensor.matmul(
                                sps[:, 0:W],
                                wTpeer[:, j * 128:(j + 1) * 128],
                                wTown[h][:, j * 128:TQ],
                                start=True, stop=True)
                            nc.vector.tensor_add(
                                sps[:, 0:128], sps[:, 0:128],
                                dmaskT_t[:, 1, :])
                            pt = pp_p.tile([128, 512], bf16, tag="pp")
                            nc.scalar.activation(pt[:, 0:W], sps[:, 0:W],
                                                 AF.Exp, scale=SCALE)
                            pts.append(pt)
                        sep = psA.tile([128, 512], f32, tag="psA", name="sep")
                        for j in range(NT):
                            W = TQ - j * 128
                            nc.tensor.matmul(
                                sep[:, j * 128:TQ], ones_col[:],
                                pts[j][:, 0:W],
                                start=(j == 0), stop=(j == NT - 1))
                        se_tot = se_p.tile([128, 512], bf16, tag="setot",
                                           bufs=2)
                        nc.vector.tensor_add(se_tot[:], own_se[h][:], sep[:])
                        bc = psB.tile([128, 512], f32, tag="psB", name="bc")
                        nc.tensor.matmul(bc[:], ones_row[:], se_tot[:],
                                         start=True, stop=True)
                        rsT = se_p.tile([128, 512], f32, tag="rsT", bufs=2)
                        nc.vector.reciprocal(rsT[:], bc[:])
                        avp = psB.tile([128, 512], f32, tag="psB", name="avp")
                        for j in range(NT):
                            W = TQ - j * 128
                            nc.tensor.matmul(
                                avp[:, j * 128:TQ],
                                wallp[:, j, :],
                                pts[j][:, 0:W],
                                start=(j == 0), stop=(j == NT - 1))
                        tmp = se_p.tile([128, 512], f32, tag="tmpo", bufs=2)
                        nc.vector.tensor_add(tmp[:], own_o[h][:], avp[:])
                        osb = ot_p.tile([128, 512], bf16, tag="ot")
                        nc.vector.tensor_mul(osb[:], tmp[:], rsT[:])
                        oT_final.append(osb)

                    # ---- cproj + residual add ----
                    for t in range(NT):
                        for ch in range(2):
                            cps = psB.tile([128, 512], f32, tag="psB",
                                           name="cps")
                            for e in range(NE):
                                nc.tensor.matmul(
                                    cps[:, 0:384],
                                    oT_final[e][:, t * 128:(t + 1) * 128],
                                    cprojT_sb[e][:, ch * 384:(ch + 1) * 384],
                                    start=(e == 0), stop=(e == NE - 1))
                            nc.vector.tensor_add(
                                x_t[t][:, ch * 384:(ch + 1) * 384],
                                x_t[t][:, ch * 384:(ch + 1) * 384],
                                cps[:, 0:384])

                    # ---- ODL ----
                    r_od = rmsnorm_stats(x_t, E, sm_p, sc_p)
                    h2_tiles = []
                    for t in range(NT):
                        h2 = hh_p.tile([128, E], bf16, tag="hh")
                        nc.scalar.activation(h2[:], x_t[t][:], AF.Copy,
                                             scale=r_od[t][:])
                        h2_tiles.append(h2)
                    h2T = []
                    for e in range(NE):
                        tp = psT.tile([128, TQ], bf16, tag="psT")
                        for t in range(NT):
                            nc.tensor.transpose(
                                tp[:, t * 128:(t + 1) * 128],
                                h2_tiles[t][:, e * 128:(e + 1) * 128],
                                ident_bf[:])
                        hsb = ht_p.tile([128, TQ], bf16, tag="ht")
                        nc.vector.tensor_copy(hsb[:], tp[:])
                        h2T.append(hsb)

                    for jc in range(NJC):
                        dtiles = []
                        for e in range(NE):
                            dt_ = wd_p.tile([128, 512], bf16, tag="wd")
                            nc.sync.dma_start(
                                dt_[:],
                                d_dencT[i, e * 128:(e + 1) * 128,
                                        jc * 512:(jc + 1) * 512])
                            dtiles.append(dt_)
                        ddtiles = []
                        for jt in range(4):
                            ddt = wdd_p.tile([128, E], bf16, tag="wdd")
                            nc.sync.dma_start(
                                ddt[:],
                                d_ddecT[i, jc * 512 + jt * 128:
                                        jc * 512 + (jt + 1) * 128, :])
                            ddtiles.append(ddt)
                        aT = []
                        for jt in range(4):
                            aps = psA.tile([128, 512], f32, tag="psA",
                                           name="aps")
                            for e in range(NE):
                                nc.tensor.matmul(
                                    aps[:],
                                    dtiles[e][:, jt * 128:(jt + 1) * 128],
                                    h2T[e][:],
                                    start=(e == 0), stop=(e == NE - 1))
                            asb = at_p.tile([128, TQ], bf16, tag="at")
                            jgl = jc * 4 + jt
                            nc.scalar.activation(
                                asb[:], aps[:], AF.Relu,
                                bias=thrneg_t[:, i, jgl:jgl + 1])
                            aT.append(asb)
                        for t in range(NT):
                            for ch in range(2):
                                dps = psB.tile([128, 512], f32, tag="psB",
                                               name="dps")
                                for jt in range(4):
                                    nc.tensor.matmul(
                                        dps[:, 0:384],
                                        aT[jt][:, t * 128:(t + 1) * 128],
                                        ddtiles[jt][:, ch * 384:(ch + 1) * 384],
                                        start=(jt == 0), stop=(jt == 3))
                                nc.vector.tensor_add(
                                    x_t[t][:, ch * 384:(ch + 1) * 384],
                                    x_t[t][:, ch * 384:(ch + 1) * 384],
                                    dps[:, 0:384])

                # ---------------- final rmsnorm -> hfT ----------------
                r_f = rmsnorm_stats(x_t, E, sm_p, sc_p)
                hf_tiles = []
                for t in range(NT):
                    hf = hh_p.tile([128, E], bf16, tag="hh")
                    nc.scalar.activation(hf[:], x_t[t][:], AF.Copy,
                                         scale=r_f[t][:])
                    hf_tiles.append(hf)
                for e in range(NE):
                    tp = psT.tile([128, TQ], bf16, tag="psT")
                    for t in range(NT):
                        nc.tensor.transpose(
                            tp[:, t * 128:(t + 1) * 128],
                            hf_tiles[t][:, e * 128:(e + 1) * 128],
                            ident_bf[:])
                    nc.vector.tensor_copy(hfT[e][:], tp[:])

            # ---------------- lm head ----------------
            with ExitStack() as mes:
                lmw_p = mes.enter_context(tc.tile_pool(name="lmw", bufs=18))
                lg_p = mes.enter_context(tc.tile_pool(name="lg", bufs=4))
                for vc, (vs, vw) in enumerate(VCH[:VCH_RUN]):
                    ltiles = []
                    for e in range(NE):
                        lw = lmw_p.tile([128, 512], bf16, tag="lmw")
                        nc.sync.dma_start(
                            lw[:, 0:vw],
                            d_lmT[e * 128:(e + 1) * 128, vs:vs + vw])
                        ltiles.append(lw)
                    for t in range(NT):
                        lps = psA.tile([128, 512], f32, tag="psA", name="lps")
                        for e in range(NE):
                            nc.tensor.matmul(
                                lps[:, 0:vw],
                                hfT[e][:, t * 128:(t + 1) * 128],
                                ltiles[e][:, 0:vw],
                                start=(e == 0), stop=(e == NE - 1))
                        lg = lg_p.tile([128, 512], bf16, tag="lg")
                        nc.vector.tensor_copy(lg[:, 0:vw], lps[:, 0:vw])
                        nc.sync.dma_start(
                            d_logits[t * 128:(t + 1) * 128, vs:vs + vw],
                            lg[:, 0:vw])

    nc.compile()
    return nc


def kernel(**inputs):
    global LAST_RESULT
    from concourse.bass_utils import run_bass_kernel_spmd

    prep, per_core = _host_prep(inputs)
    key = prep["trivial_blend"]
    if key not in _NC_CACHE:
        _NC_CACHE[key] = _build_nc(key)
    nc = _NC_CACHE[key]

    in_maps = []
    for c in range(N_CORES):
        pc = per_core[c]
        in_maps.append({
            "xemb": pc["xemb"], "cosr": pc["cosr"], "sinr": pc["sinr"],
            "dmaskT": pc["dmaskT"],
            "qkvT": prep["qkvT"], "cprojT": prep["cprojT"],
            "dencT": prep["dencT"], "ddecT": prep["ddecT"],
            "lmT": prep["lmT"], "thrneg": prep["thrneg"],
            "lamr": prep["lamr"], "lamx": prep["lamx"],
        })
    trace = bool(_os.environ.get("KBENCH_TRACE"))
    res = run_bass_kernel_spmd(nc, in_maps, core_ids=list(range(N_CORES)),
                               trace=trace,
                               trace_cores=list(range(N_CORES)) if trace else None)
    LAST_RESULT = res
    out = np.empty((B, T, V), dtype=np.float32)
    for c in range(N_CORES):
        b, half = c // 2, c % 2
        out[b, _own_rows(half)] = np.asarray(
            res.results[c]["logits"]).astype(np.float32)
    return out
